# revision 5
# baseline (speedup 1.0000x reference)
"""Trainium2 Bass kernel for nn_CubicalModel_ISM.

Reference computation:
    Xp = reshape(I1 @ p0, (28, 28)); Yp = reshape(I2 @ p1, (28, 28))
    dgm1 = Xp[inds1[0::2], inds1[1::2]].reshape(50, 2)
    dgm2 = Yp[inds2[0::2], inds2[1::2]].reshape(50, 2)

Only the <=100 gathered rows of each 784-row GEMV are live, and the gather
commutes with the per-row dot product.  So the host selects the 100 indexed
rows of I1 and of I2 (the "tiny gather", applied to the input instead of the
output), the device computes the 200 surviving dot products of length 32768
with k sharded over the 8 cores (3.3 MB of HBM traffic per core), and the
host sums the 8 partial vectors (the k-unshard) and reshapes.

Per-core device program (identical on all cores, data differs):
    a [4096, 202] f32 : this core's k-slice, transposed so k lands on SBUF
                        partitions.  Columns 0..99 = selected I1 rows,
                        100..199 = selected I2 rows, 200 = p0 chunk,
                        201 = p1 chunk (packing p into the same tile keeps
                        every matmul to a single DMA semaphore wait — the
                        PE LoadWeights struct only supports one).
    y [128, 2]    f32 : y[:100,0] / y[:100,1] = partial dot products

    for c in 32 k-chunks:  (PE, matrix chunk stationary, 1 moving column)
        ps1[100,1] += a[c][:, 0:100].T   @ a[c][:, 200:201]
        ps2[100,1] += a[c][:, 100:200].T @ a[c][:, 201:202]
"""

import numpy as np

K = 32768
NCORES = 8
KS = K // NCORES          # 4096 k columns per core
NCH = KS // 128           # 32 k-chunks of 128 per core
R = 100                   # gathered rows per diagram
W = 2 * R + 2             # tile width: 200 matrix columns + p0 + p1
SIDE = 28

_cache = {}


def _build_nc():
    import concourse.bacc as bacc
    import concourse.mybir as mybir
    from concourse.tile import TileContext

    f32 = mybir.dt.float32
    nc = bacc.Bacc("TRN2", target_bir_lowering=False, debug=False,
                   num_devices=NCORES)
    a = nc.declare_dram_parameter("a", [KS, W], f32, isOutput=False)
    y = nc.declare_dram_parameter("y", [128, 2], f32, isOutput=True)

    with TileContext(nc) as tc:
        with (
            tc.tile_pool(name="apool", bufs=NCH) as apool,
            tc.tile_pool(name="opool", bufs=1) as opool,
            tc.tile_pool(name="ps", bufs=1, space="PSUM") as pspool,
        ):
            ps1 = pspool.tile([R, 1], f32)
            ps2 = pspool.tile([R, 1], f32)
            for c in range(NCH):
                at = apool.tile([128, W], f32)
                nc.sync.dma_start(out=at, in_=a[c * 128:(c + 1) * 128, :])
                nc.tensor.matmul(
                    ps1, at[:, 0:R], at[:, 2 * R:2 * R + 1],
                    start=(c == 0), stop=(c == NCH - 1),
                )
                nc.tensor.matmul(
                    ps2, at[:, R:2 * R], at[:, 2 * R + 1:2 * R + 2],
                    start=(c == 0), stop=(c == NCH - 1),
                )

            yt = opool.tile([128, 2], f32)
            nc.vector.memset(yt, 0.0)
            nc.vector.tensor_copy(out=yt[0:R, 0:1], in_=ps1)
            nc.vector.tensor_copy(out=yt[0:R, 1:2], in_=ps2)
            nc.sync.dma_start(out=y[:], in_=yt)
    nc.compile()
    return nc


def _prep_inputs(p0, p1, I1, I2, inds1, inds2):
    idx1 = inds1.astype(np.int64).reshape(-1, 2)
    idx2 = inds2.astype(np.int64).reshape(-1, 2)
    rows1 = idx1[:, 0] * SIDE + idx1[:, 1]      # flat positions, in order
    rows2 = idx2[:, 0] * SIDE + idx2[:, 1]

    selT = np.empty((K, W), np.float32)
    selT[:, 0:R] = I1[rows1, :].T
    selT[:, R:2 * R] = I2[rows2, :].T
    selT[:, 2 * R] = p0
    selT[:, 2 * R + 1] = p1

    return [{"a": selT[cix * KS:(cix + 1) * KS]} for cix in range(NCORES)]


def _run(in_maps, trace=False):
    from concourse.bass_utils import run_bass_kernel_spmd

    if "nc" not in _cache:
        _cache["nc"] = _build_nc()
    return run_bass_kernel_spmd(
        _cache["nc"], in_maps, list(range(NCORES)), trace=trace
    )


def kernel(p0, p1, I1, I2, inds1, inds2):
    p0 = np.ascontiguousarray(np.asarray(p0, dtype=np.float32))
    p1 = np.ascontiguousarray(np.asarray(p1, dtype=np.float32))
    I1 = np.asarray(I1, dtype=np.float32)
    I2 = np.asarray(I2, dtype=np.float32)
    inds1 = np.asarray(inds1)
    inds2 = np.asarray(inds2)

    in_maps = _prep_inputs(p0, p1, I1, I2, inds1, inds2)
    results = _run(in_maps).results

    acc = np.zeros((R, 2), np.float64)
    for r in results:
        acc += r["y"][:R, :].astype(np.float64)
    vals = acc.astype(np.float32)
    dgm1 = vals[:, 0].reshape(R // 2, 2)
    dgm2 = vals[:, 1].reshape(R // 2, 2)
    return (dgm1, dgm2)


# revision 7
# speedup vs baseline: 1.4032x; 1.4032x over previous
"""Trainium2 Bass kernel for nn_CubicalModel_ISM.

Reference computation:
    Xp = reshape(I1 @ p0, (28, 28)); Yp = reshape(I2 @ p1, (28, 28))
    dgm1 = Xp[inds1[0::2], inds1[1::2]].reshape(50, 2)
    dgm2 = Yp[inds2[0::2], inds2[1::2]].reshape(50, 2)

Only the <=100 gathered rows of each 784-row GEMV are live, and the gather
commutes with the per-row dot product.  So the host selects the 100 indexed
rows of I1 and of I2 (the "tiny gather", applied to the input instead of the
output), the device computes the 200 surviving dot products of length 32768
with k sharded over the 8 cores (3.3 MB of HBM traffic per core), and the
host sums the 8 partial vectors (the k-unshard) and reshapes.

Precision/speed: plain fp32 matmuls stream at 1/4 PE rate and fp32r loses
~1e-4; instead every fp32 operand is split hi+lo into two fp16 halves
(22 mantissa bits total) and the product expanded as
    A.q ~= Ahi.qhi + Ahi.qlo + Alo.qhi        (the lo.lo term is ~2^-22)
with all three terms as full-rate fp16 matmuls accumulating into fp32 PSUM.
Same total HBM bytes as fp32 (2 x 2-byte halves).

Per-core device program: the core's 4096 k-rows are split into 16 tiles of
two 128-row k-chunks.  A tile packs, per SBUF partition p (k within chunk):

    cols   0:200  hi halves, chunk 2t   (I1 rows 100 | I2 rows 100)
    cols 200:400  hi halves, chunk 2t+1
    cols 400:600  lo halves, chunk 2t
    cols 600:800  lo halves, chunk 2t+1
    cols 800:804  q hi: p0[2t], p1[2t], p0[2t+1], p1[2t+1]
    cols 804:808  q lo: same order

Two matmuls per tile, all 32 accumulating into one PSUM [8, 400] tile:
    mA: lhsT = q hi+lo (8 cols), rhs = hi matrix (400 cols)
        rows 0-3 += qhi.Ahi       rows 4-7 += qlo.Ahi
    mB: lhsT = q hi (4 cols),    rhs = lo matrix (400 cols)
        rows 0-3 += qhi.Alo
Useful segments (j = gathered-row index, even/odd chunk halves):
    dgm1 partials: rows {0,4}[0:100]   and rows {2,6}[200:300]
    dgm2 partials: rows {1,5}[100:200] and rows {3,7}[300:400]
Off-segment entries accumulate garbage cross-terms; never read.  The host
adds the segments and reduces across the 8 cores.  DMA issue alternates
between the two HWDGE engines (SP and ACT) so descriptor submission is not
serialized on one sequencer.
"""

import numpy as np

K = 32768
NCORES = 8
KS = K // NCORES          # 4096 k columns per core
T = KS // 256             # 16 tiles of 2 k-chunks
TW = 808                  # 8 blocks of 100 fp16 matrix cols + 8 q cols
R = 100                   # gathered rows per diagram
SIDE = 28

_cache = {}


def _build_nc():
    import concourse.bacc as bacc
    import concourse.mybir as mybir
    from concourse.tile import TileContext

    f32 = mybir.dt.float32
    f16 = mybir.dt.float16
    nc = bacc.Bacc("TRN2", target_bir_lowering=False, debug=False,
                   num_devices=NCORES)
    a = nc.declare_dram_parameter("a", [T, 128, TW], f16, isOutput=False)
    y = nc.declare_dram_parameter("y", [8, 400], f32, isOutput=True)

    with TileContext(nc) as tc:
        with (
            tc.tile_pool(name="apool", bufs=T) as apool,
            tc.tile_pool(name="opool", bufs=1) as opool,
            tc.tile_pool(name="ps", bufs=1, space="PSUM") as pspool,
        ):
            ps = pspool.tile([8, 400], f32)
            for t in range(T):
                at = apool.tile([128, TW], f16)
                eng = nc.sync if t % 2 == 0 else nc.scalar
                eng.dma_start(out=at, in_=a[t])

                def mA(start=False, stop=False):
                    nc.tensor.matmul(ps, at[:, 800:808], at[:, 0:400],
                                     start=start, stop=stop)

                def mB():
                    nc.tensor.matmul(ps[0:4, :], at[:, 800:804],
                                     at[:, 400:800], start=False, stop=False)

                if t == 0:
                    mA(start=True)
                    mB()
                elif t == T - 1:
                    mB()
                    mA(stop=True)
                else:
                    mA()
                    mB()

            yt = opool.tile([8, 400], f32)
            nc.vector.tensor_copy(out=yt, in_=ps)
            nc.sync.dma_start(out=y[:], in_=yt)
    nc.compile()
    return nc


def _split16(x):
    hi = x.astype(np.float16)
    lo = (x - hi.astype(np.float32)).astype(np.float16)
    return hi, lo


def _prep_inputs(p0, p1, I1, I2, inds1, inds2):
    idx1 = inds1.astype(np.int64).reshape(-1, 2)
    idx2 = inds2.astype(np.int64).reshape(-1, 2)
    rows1 = idx1[:, 0] * SIDE + idx1[:, 1]      # flat positions, in order
    rows2 = idx2[:, 0] * SIDE + idx2[:, 1]

    selT = np.empty((K, 2 * R), np.float32)
    selT[:, 0:R] = I1[rows1, :].T
    selT[:, R:2 * R] = I2[rows2, :].T
    sel_hi, sel_lo = _split16(selT)             # [K, 200] each
    q = np.stack([p0, p1], axis=-1)             # [K, 2]
    q_hi, q_lo = _split16(q)

    in_maps = []
    for cix in range(NCORES):
        o = cix * KS
        bh = sel_hi[o:o + KS].reshape(T, 2, 128, 2 * R)
        bl = sel_lo[o:o + KS].reshape(T, 2, 128, 2 * R)
        qh = q_hi[o:o + KS].reshape(T, 2, 128, 2)
        ql = q_lo[o:o + KS].reshape(T, 2, 128, 2)
        a = np.empty((T, 128, TW), np.float16)
        a[:, :, 0:200] = bh[:, 0]
        a[:, :, 200:400] = bh[:, 1]
        a[:, :, 400:600] = bl[:, 0]
        a[:, :, 600:800] = bl[:, 1]
        a[:, :, 800:802] = qh[:, 0]
        a[:, :, 802:804] = qh[:, 1]
        a[:, :, 804:806] = ql[:, 0]
        a[:, :, 806:808] = ql[:, 1]
        in_maps.append({"a": a})
    return in_maps


def _run(in_maps, trace=False):
    from concourse.bass_utils import run_bass_kernel_spmd

    if "nc" not in _cache:
        _cache["nc"] = _build_nc()
    return run_bass_kernel_spmd(
        _cache["nc"], in_maps, list(range(NCORES)), trace=trace
    )


def kernel(p0, p1, I1, I2, inds1, inds2):
    p0 = np.ascontiguousarray(np.asarray(p0, dtype=np.float32))
    p1 = np.ascontiguousarray(np.asarray(p1, dtype=np.float32))
    I1 = np.asarray(I1, dtype=np.float32)
    I2 = np.asarray(I2, dtype=np.float32)
    inds1 = np.asarray(inds1)
    inds2 = np.asarray(inds2)

    in_maps = _prep_inputs(p0, p1, I1, I2, inds1, inds2)
    results = _run(in_maps).results

    acc = np.zeros((2, R), np.float64)
    for r in results:
        yc = r["y"].astype(np.float64)
        acc[0] += (yc[0, 0:100] + yc[4, 0:100]
                   + yc[2, 200:300] + yc[6, 200:300])      # dgm1 partials
        acc[1] += (yc[1, 100:200] + yc[5, 100:200]
                   + yc[3, 300:400] + yc[7, 300:400])      # dgm2 partials
    vals = acc.astype(np.float32)
    dgm1 = vals[0].reshape(R // 2, 2)
    dgm2 = vals[1].reshape(R // 2, 2)
    return (dgm1, dgm2)
